# revision 11
# baseline (speedup 1.0000x reference)
"""Exponential smoothing (per-channel EMA over time) on 8 Trainium2 cores.

  s_0 = x_0 ; s_t = a * x_t + (1 - a) * s_{t-1},  a = sigmoid(alpha)  (per channel)

Full shapes: x (16, 4096, 512) f32, alpha (1, 1, 512) f32 -> out (16, 4096, 512).
Sharding: data-parallel over batch B (16 -> 2 per core); alpha replicated.

Per core, per 2048-step time chunk:
  1. DMA-loads x (cast to bf16 on host) in native layout (t on partitions)
     via the Sync HWDGE queue; alpha rides the Scalar HWDGE queue in parallel
     so the x stream starts immediately after the program prologue.
  2. Transposes 128x128 blocks on the tensor engine into 2-bank PSUM tiles
     (time on the free axis, channels on partitions).
  3. Runs a hand-built custom DVE op (EMA_PAGED_ANT, registered below) that
     scans r_t = w*r_{t-1} + x_t directly out of PSUM at ~1.1 cyc/element,
     writing bf16 r to SBUF. The scan is the rescaled form r = s/a, so no
     pre-scale pass is needed; chunk chaining passes the previous chunk's
     last column as the carry. Chunk 0 seeds with r_{-1} = x_0/a, making
     s_0 = x_0 exactly. The first chunk is split (256+1792) so the vector
     engine starts as soon as the first 256KB of x lands; the last chunk is
     split in halves so the tail back-transposes overlap the final scans.
  4. Transposes back via a regular matmul against diag(a) (contracting the
     channel partitions), so s = a*r is applied by the tensor engine for free.
  5. Scalar engine evacuates PSUM -> bf16 SBUF; stores to HBM alternate
     between the GpSimd SWDGE queue and the Sync HWDGE queue so the output
     stream is never bound by a single ~210 GB/s queue.
Host converts the bf16 y back to f32.
"""

from contextlib import ExitStack

import ml_dtypes
import numpy as np

import concourse.bass as bass
import concourse.dve_ops as dve_ops
import concourse.tile as tile
from concourse import bacc, mybir
from concourse.bass_utils import run_bass_kernel_spmd
from concourse.dve_ops import DveOp
from concourse.dve_spec import (
    C0,
    C1,
    AluOp,
    Bin,
    One,
    Spec,
    Src0,
    Src1,
    _Placement,
    _Stage,
    _State,
    _assemble,
    COUNT_ONCE,
    PREV,
)
from concourse.dve_uop import AluInp, DveOpSpec, OutSel, Trigger
from concourse.masks import make_identity

# ---------------------------------------------------------------------------
# Custom DVE op: paged EMA scan, r_k = w*r_{k-1} + u_k at ~1.1 cyc/element.
# Within each 16-element page the weights are formed as w^(i+1) * w^-(j+1)
# via three no-bubble running products/sums; two non-consuming bubble uOps at
# each page boundary rescale the carry by w^16 exactly. fp32 range needs
# w^-15 finite: OK for w >= sigmoid(-5.8).
# ---------------------------------------------------------------------------

CURR = AluInp.CURR_ALU_OUT
SWAP = AluInp.CURR_SWAP_OUT
LANE_M = AluInp.PREV_DELAY_3
LANE_R = AluInp.PREV_DELAY_4
PAGE = 16
_BYP = _Stage(AluOp.BYPASS, PREV)


def _build_ema_uops():
    m_key = Bin(AluOp.MULTIPLY, Src0, C1)
    r_key = Bin(AluOp.MULTIPLY, C0, C0)
    p = _Placement(
        pipeline=[
            _Stage(AluOp.MULTIPLY, CURR, C1),     # st0: Inv <- Inv * (1/w)
            _Stage(AluOp.MULTIPLY, Src0, PREV),   # st1: m = u * Inv
            _Stage(AluOp.MULTIPLY, CURR, C0),     # st2: R <- R * w
            _Stage(AluOp.ADD, CURR, LANE_M),      # st3: A <- A + m
            _Stage(AluOp.MULTIPLY, PREV, LANE_R), # st4: out = A * R
            _BYP, _BYP, _BYP,
        ],
        node_stage={},
        lane={Src0: 0, C0: 1, C1: 2, m_key: 3, r_key: 4, One: 5},
        out_sel=OutSel.ALU_OUT,
        accum_stage=None,
        captures=[(2, 3), (3, 4)],
    )
    latch_p = _Placement(
        pipeline=[_BYP] * 8, node_stage={}, lane={Src1: 0},
        out_sel=OutSel.ALU_OUT, accum_stage=None, captures=[],
    )
    states = [
        _State(  # 0: latch-init — park the carry (in1) in st3's swap flop
            placement=latch_p, trigger=COUNT_ONCE, repeat=1,
            consume=(False, True),
            overrides={3: _Stage(AluOp.BYPASS, Src1, Src1, swap=True)},
            write_out=False, next=(1, 0, 0),
        ),
        _State(  # 1: seed — Inv=1, R=1, A=carry
            placement=p, trigger=COUNT_ONCE, repeat=1, consume=(False, False),
            overrides={
                0: _Stage(AluOp.BYPASS, One),
                2: _Stage(AluOp.BYPASS, One),
                3: _Stage(AluOp.BYPASS, SWAP),
            },
            write_out=False, next=(2, 0, 0),
        ),
        _State(  # 2: steady — 1 element/cycle; page wrap -> bubbles
            placement=p,
            trigger=(Trigger.SRC_TENSOR_DONE, Trigger.SUB_DIM_DONE, Trigger.NONE),
            next=(0, 3, 0), repeat=0, consume=(True, False), write_out=True,
        ),
        _State(  # 3: B1 — A <- A * w^PAGE (R held at st2)
            placement=p, trigger=COUNT_ONCE, repeat=1, consume=(False, False),
            overrides={
                0: _Stage(AluOp.BYPASS, CURR), 1: _BYP,
                2: _Stage(AluOp.BYPASS, CURR),
                3: _Stage(AluOp.MULTIPLY, CURR, PREV), 4: _BYP,
            },
            write_out=False, next=(4, 0, 0),
        ),
        _State(  # 4: B2 — reset Inv/R for the new page, hold A
            placement=p, trigger=COUNT_ONCE, repeat=1, consume=(False, False),
            overrides={
                0: _Stage(AluOp.BYPASS, One), 1: _BYP,
                2: _Stage(AluOp.BYPASS, One),
                3: _Stage(AluOp.BYPASS, CURR), 4: _BYP,
            },
            write_out=False, next=(2, 0, 0),
        ),
    ]
    uops = [_assemble(s) for s in states]
    for u in uops:
        u.validate("v3")
    return uops


def _ema_ref(in0, in1, s0, s1, imm2):
    P = in0.shape[0]
    u = in0.astype(np.float64).reshape(P, -1)
    w = np.asarray(s0, np.float64).reshape(P, 1)
    r = np.asarray(in1, np.float64).reshape(P, 1)[:, 0].copy()
    out = np.empty_like(u)
    for t in range(u.shape[1]):
        r = w[:, 0] * r + u[:, t]
        out[:, t] = r
    return out.reshape(in0.shape).astype(np.float32)


class _HandDveOp(DveOp):
    def compile(self, ver):
        if ver != "v3":
            raise ValueError(f"{self.name}: hand-built for v3/TRN2 only")
        cached = dve_ops._COMPILE_CACHE.get((self.name, ver))
        if cached is not None:
            return cached
        spec = DveOpSpec(
            name=self.name,
            opcode=dve_ops.get_dve_sub_opcode(self.name),
            uops=_build_ema_uops(),
            rd1_en=True,
        )
        dve_ops._COMPILE_CACHE[(self.name, ver)] = spec
        return spec


def _register_ema_op() -> DveOp:
    for op in dve_ops.OPS:
        if op.name == "EMA_PAGED_ANT":
            return op
    op = _HandDveOp(
        "EMA_PAGED_ANT",
        Spec(
            body=Bin(AluOp.ADD, Bin(AluOp.MULTIPLY, Src0, C1),
                     Bin(AluOp.MULTIPLY, Src1, C0)),
            reference=_ema_ref,
        ),
        subdim=True,
        uops_sha={},
    )
    dve_ops.OPS.append(op)
    dve_ops.CUSTOM_DVE_SPECS[op.name] = op.spec
    dve_ops._SUB_OPCODE_FOR_NAME[op.name] = (
        dve_ops._CUSTOM_DVE_ROW_BASE + len(dve_ops.OPS) - 1
    )
    assert dve_ops._SUB_OPCODE_FOR_NAME[op.name] < 0x20
    return op


# ---------------------------------------------------------------------------
# Kernel
# ---------------------------------------------------------------------------

B, T, D = 16, 4096, 512
NCORES = 8
BL = B // NCORES   # batches per core
P = 128            # partitions
TCW = 2048         # time chunk per pipeline iteration
ND = D // P        # channel chunks of 128 (4)
NK = TCW // P      # 128-row sub-chunks per time chunk (16)

FP32 = mybir.dt.float32
BF16 = mybir.dt.bfloat16


def build_program(bl: int = BL, t: int = T) -> bacc.Bacc:
    ema = _register_ema_op()
    nc = bacc.Bacc(
        "TRN2",
        target_bir_lowering=False,
        debug=False,
        enable_asserts=False,
        num_devices=NCORES,
    )
    x = nc.dram_tensor("x", (bl, t, D), BF16, kind="ExternalInput").ap()
    # Host-precomputed per-channel tables: (w | 1/w | 1/a | a), channel
    # d = j*128 + p laid out as [128, 4*ND].  Avoids the sigmoid
    # activation-table-load + reciprocal chain at startup.
    wt = nc.dram_tensor("wt", (P, 4 * ND), FP32, kind="ExternalInput").ap()
    y = nc.dram_tensor("y", (bl, t, D), BF16, kind="ExternalOutput").ap()

    with tile.TileContext(nc) as tc, ExitStack() as ctx:
        const_pool = ctx.enter_context(tc.tile_pool(name="const", bufs=1))
        xn_pool = ctx.enter_context(tc.tile_pool(name="xn", bufs=3))
        pin_pool = ctx.enter_context(tc.tile_pool(name="pin", bufs=2, space="PSUM"))
        pout_pool = ctx.enter_context(tc.tile_pool(name="pout", bufs=2, space="PSUM"))
        s_pool = ctx.enter_context(tc.tile_pool(name="s", bufs=18))
        y_pool = ctx.enter_context(tc.tile_pool(name="y", bufs=3))
        carry_pool = ctx.enter_context(tc.tile_pool(name="carry", bufs=1))

        ident = const_pool.tile([P, P], BF16)
        make_identity(nc, ident[:])

        # Per-channel tables ride the Scalar HWDGE queue so the Sync queue
        # starts streaming x immediately.
        wt_sb = const_pool.tile([P, 4 * ND], FP32)
        nc.scalar.dma_start(wt_sb[:], wt)
        w_sb = wt_sb[:, 0:ND]
        inv_w = wt_sb[:, ND : 2 * ND]
        inv_a = wt_sb[:, 2 * ND : 3 * ND]
        a_sb = wt_sb[:, 3 * ND : 4 * ND]

        # diag(a) per channel chunk: ident row p scaled by a[p] (bf16), on
        # the scalar engine (cheap per-partition scale).
        diags = []
        for j in range(ND):
            dg = const_pool.tile([P, P], BF16, tag=f"diag{j}")
            nc.scalar.mul(dg[:], ident[:], a_sb[:, j : j + 1])
            diags.append(dg)

        inits = carry_pool.tile([P, bl * ND], FP32)

        def back_chunk(ss, b, t0, m0, m1, yout, evac_engines=(nc.scalar,)):
            """Back-transpose (diag(a) matmul) + PSUM evac for k-pairs
            m0..m1 of the chunk whose scan outputs are `ss`."""
            for m in range(m0, m1):
                pout = pout_pool.tile([P, 2 * D], FP32, tag="pout")
                for h in range(2):
                    k = 2 * m + h
                    for j in range(ND):
                        nc.tensor.matmul(
                            pout[:, h * D + j * P : (h * D + (j + 1) * P)],
                            ss[j][:, k * P : (k + 1) * P],
                            diags[j][:],
                        )
                eng = evac_engines[m % len(evac_engines)]
                if eng is nc.vector:
                    nc.vector.tensor_copy(yout[:, 2 * m : 2 * m + 2, :], pout[:])
                else:
                    eng.copy(yout[:, 2 * m : 2 * m + 2, :], pout[:])

        def out_dma(eng, b, t0, r0, r1, yout):
            eng.dma_start(
                y[b, t0 + r0 : t0 + r1, :].rearrange("(k p) d -> p k d", p=P),
                yout[:, r0 // P : r1 // P, :],
            )

        # Chunk schedule: (b, t0, clen).  The leading 256-step chunks per
        # batch start the vector engine as soon as the first 256KB of x
        # lands (the inits mul then only depends on 2 transposes).
        chunks = [
            (0, 0, 256),
            (0, 256, 1792),
            (1, 0, 256),
            (1, 256, 1792),
            (0, 2048, 2048),
            (1, 2048, 2048),
        ]
        s_prevs = [None] * bl
        pending = None  # deferred back-pass: (ss, b, t0, clen)
        for ci, (b, t0, clen) in enumerate(chunks):
            last = ci == len(chunks) - 1
            nk = clen // P
            xn = xn_pool.tile([P, NK, D], BF16, tag="xn")
            # Input pieces (Sync HWDGE, input-only so never blocked):
            if clen <= 512:
                pieces = (0, clen)
            else:
                pieces = (0, clen // 2, clen)
            for r0, r1 in zip(pieces[:-1], pieces[1:]):
                nc.sync.dma_start(
                    xn[:, r0 // P : r1 // P, :],
                    x[b, t0 + r0 : t0 + r1, :].rearrange("(k p) d -> p k d", p=P),
                )

            ss = []
            scan_args = []
            for j in range(ND):
                pin = pin_pool.tile([P, TCW], BF16, tag="pin")
                for k in range(nk):
                    nc.tensor.transpose(
                        pin[:, k * P : (k + 1) * P],
                        xn[:, k, j * P : (j + 1) * P],
                        ident[:],
                    )
                if t0 == 0:
                    # carry r_{-1} = x_0 / a  =>  s_0 = x_0 exactly
                    nc.scalar.mul(
                        inits[:, b * ND + j : b * ND + j + 1],
                        pin[:, 0:1],
                        inv_a[:, j : j + 1],
                    )
                    carry = inits[:, b * ND + j : b * ND + j + 1]
                else:
                    prev_s, prev_len = s_prevs[b]
                    carry = prev_s[j][:, prev_len - 1 : prev_len]
                s = s_pool.tile([P, TCW], BF16, tag="s", name=f"s_{b}_{t0}_{j}")
                ss.append(s)
                scan_args.append((pin, carry, s))

            def emit_scans(c0, c1):
                for j, (pin, carry, s) in enumerate(scan_args):
                    cr = carry if c0 == 0 else s[:, c0 - 1 : c0]
                    nc.vector._custom_dve(
                        ema,
                        out=s[:, c0:c1].rearrange("p (s n) -> p s n", n=PAGE),
                        in0=pin[:, c0:c1].rearrange("p (s n) -> p s n", n=PAGE),
                        in1=cr,
                        s0=w_sb[:, j : j + 1],
                        s1=inv_w[:, j : j + 1],
                    )

            if not last:
                emit_scans(0, clen)
                s_prevs[b] = (ss, clen)
                # Deferred back-pass of the PREVIOUS chunk: its scans are done
                # by now, so the in-order tensor queue never stalls ahead of
                # vector-critical transposes.
                if pending is not None:
                    pss, pb, pt0, pclen = pending
                    pnk = pclen // P
                    yout = y_pool.tile([P, NK, D], BF16, tag="y")
                    back_chunk(pss, pb, pt0, 0, pnk // 2, yout)
                    if pclen <= 512:
                        out_dma(nc.gpsimd if ci % 2 else nc.scalar, pb, pt0, 0, pclen, yout)
                    else:
                        out_dma(nc.gpsimd, pb, pt0, 0, pclen // 2, yout)
                        out_dma(nc.scalar, pb, pt0, pclen // 2, pclen, yout)
                pending = (ss, b, t0, clen)
            else:
                # Last chunk: scans in halves, back-pass chasing each half
                # with evacs split across scalar+vector, so the tail drains
                # as fast as possible.
                emit_scans(0, clen // 2)
                pss, pb, pt0, pclen = pending
                pnk = pclen // P
                pyout = y_pool.tile([P, NK, D], BF16, tag="y")
                back_chunk(pss, pb, pt0, 0, pnk // 2, pyout)
                out_dma(nc.gpsimd, pb, pt0, 0, pclen // 2, pyout)
                out_dma(nc.scalar, pb, pt0, pclen // 2, pclen, pyout)
                emit_scans(clen // 2, clen)
                yout = y_pool.tile([P, NK, D], BF16, tag="y")
                back_chunk(ss, b, t0, 0, nk // 4, yout,
                           evac_engines=(nc.scalar, nc.vector))
                out_dma(nc.gpsimd, b, t0, 0, clen // 2, yout)
                back_chunk(ss, b, t0, nk // 4, nk // 2, yout,
                           evac_engines=(nc.scalar, nc.vector))
                out_dma(nc.scalar, b, t0, clen // 2, clen * 3 // 4, yout)
                out_dma(nc.gpsimd, b, t0, clen * 3 // 4, clen, yout)

    nc.compile()
    return nc


_prog = None


def make_in_maps(x, alpha):
    x = np.asarray(x)
    alpha = np.asarray(alpha, dtype=np.float64)
    assert x.shape == (B, T, D) and alpha.shape == (1, 1, D)
    xb = np.ascontiguousarray(x.astype(ml_dtypes.bfloat16))
    # Per-channel tables (w | 1/w | 1/a | a) as [128, 4*ND], channel
    # d = j*128 + p (so column j holds channels j*128..j*128+127).
    a = 1.0 / (1.0 + np.exp(-alpha[0, 0, :]))  # (D,)
    w = 1.0 - a
    wt = np.empty((P, 4 * ND), np.float32)
    for bi, tb in enumerate((w, 1.0 / w, 1.0 / a, a)):
        wt[:, bi * ND : (bi + 1) * ND] = tb.reshape(ND, P).T
    return [
        {"x": np.ascontiguousarray(xb[i * BL : (i + 1) * BL]), "wt": wt}
        for i in range(NCORES)
    ]


def kernel(x, alpha):
    global _prog
    if _prog is None:
        _prog = build_program()
    in_maps = make_in_maps(x, alpha)
    res = run_bass_kernel_spmd(_prog, in_maps, core_ids=list(range(NCORES)))
    out = np.concatenate([r["y"] for r in res.results], axis=0)
    return np.ascontiguousarray(out.astype(np.float32))


# revision 15
# speedup vs baseline: 1.0834x; 1.0834x over previous
"""Exponential smoothing (per-channel EMA over time) on 8 Trainium2 cores.

  s_0 = x_0 ; s_t = a * x_t + (1 - a) * s_{t-1},  a = sigmoid(alpha)  (per channel)

Full shapes: x (16, 4096, 512) f32, alpha (1, 1, 512) f32 -> out (16, 4096, 512).
Sharding: data-parallel over batch B (16 -> 2 per core); alpha replicated.

Per core, per 2048-step time chunk:
  1. DMA-loads x (cast to bf16 on host) in native layout (t on partitions)
     via the Sync HWDGE queue; alpha rides the Scalar HWDGE queue in parallel
     so the x stream starts immediately after the program prologue.
  2. Transposes 128x128 blocks on the tensor engine into 2-bank PSUM tiles
     (time on the free axis, channels on partitions).
  3. Runs a hand-built custom DVE op (EMA_PAGED_ANT, registered below) that
     scans r_t = w*r_{t-1} + x_t directly out of PSUM at ~1.1 cyc/element,
     writing bf16 r to SBUF. The scan is the rescaled form r = s/a, so no
     pre-scale pass is needed; chunk chaining passes the previous chunk's
     last column as the carry. Chunk 0 seeds with r_{-1} = x_0/a, making
     s_0 = x_0 exactly. The first chunk is split (256+1792) so the vector
     engine starts as soon as the first 256KB of x lands; the last chunk is
     split in halves so the tail back-transposes overlap the final scans.
  4. Transposes back via a regular matmul against diag(a) (contracting the
     channel partitions), so s = a*r is applied by the tensor engine for free.
  5. Scalar engine evacuates PSUM -> bf16 SBUF; stores to HBM alternate
     between the GpSimd SWDGE queue and the Sync HWDGE queue so the output
     stream is never bound by a single ~210 GB/s queue.
Host converts the bf16 y back to f32.
"""

from contextlib import ExitStack

import ml_dtypes
import numpy as np

import concourse.bass as bass
import concourse.dve_ops as dve_ops
import concourse.tile as tile
from concourse import bacc, mybir
from concourse.bass_utils import run_bass_kernel_spmd
from concourse.dve_ops import DveOp
from concourse.dve_spec import (
    C0,
    C1,
    AluOp,
    Bin,
    One,
    Spec,
    Src0,
    Src1,
    _Placement,
    _Stage,
    _State,
    _assemble,
    COUNT_ONCE,
    PREV,
)
from concourse.dve_uop import AluInp, DveOpSpec, OutSel, Trigger
from concourse.masks import make_identity

# ---------------------------------------------------------------------------
# Custom DVE op: paged EMA scan, r_k = w*r_{k-1} + u_k at ~1.1 cyc/element.
# Within each 16-element page the weights are formed as w^(i+1) * w^-(j+1)
# via three no-bubble running products/sums; two non-consuming bubble uOps at
# each page boundary rescale the carry by w^16 exactly. fp32 range needs
# w^-15 finite: OK for w >= sigmoid(-5.8).
# ---------------------------------------------------------------------------

CURR = AluInp.CURR_ALU_OUT
SWAP = AluInp.CURR_SWAP_OUT
LANE_M = AluInp.PREV_DELAY_3
LANE_R = AluInp.PREV_DELAY_4
PAGE = 16
_BYP = _Stage(AluOp.BYPASS, PREV)


def _build_ema_uops():
    m_key = Bin(AluOp.MULTIPLY, Src0, C1)
    r_key = Bin(AluOp.MULTIPLY, C0, C0)
    p = _Placement(
        pipeline=[
            _Stage(AluOp.MULTIPLY, CURR, C1),     # st0: Inv <- Inv * (1/w)
            _Stage(AluOp.MULTIPLY, Src0, PREV),   # st1: m = u * Inv
            _Stage(AluOp.MULTIPLY, CURR, C0),     # st2: R <- R * w
            _Stage(AluOp.ADD, CURR, LANE_M),      # st3: A <- A + m
            _Stage(AluOp.MULTIPLY, PREV, LANE_R), # st4: out = A * R
            _BYP, _BYP, _BYP,
        ],
        node_stage={},
        lane={Src0: 0, C0: 1, C1: 2, m_key: 3, r_key: 4, One: 5},
        out_sel=OutSel.ALU_OUT,
        accum_stage=None,
        captures=[(2, 3), (3, 4)],
    )
    latch_p = _Placement(
        pipeline=[_BYP] * 8, node_stage={}, lane={Src1: 0},
        out_sel=OutSel.ALU_OUT, accum_stage=None, captures=[],
    )
    states = [
        _State(  # 0: latch-init — park the carry (in1) in st3's swap flop
            placement=latch_p, trigger=COUNT_ONCE, repeat=1,
            consume=(False, True),
            overrides={3: _Stage(AluOp.BYPASS, Src1, Src1, swap=True)},
            write_out=False, next=(1, 0, 0),
        ),
        _State(  # 1: seed — Inv=1, R=1, A=carry
            placement=p, trigger=COUNT_ONCE, repeat=1, consume=(False, False),
            overrides={
                0: _Stage(AluOp.BYPASS, One),
                2: _Stage(AluOp.BYPASS, One),
                3: _Stage(AluOp.BYPASS, SWAP),
            },
            write_out=False, next=(2, 0, 0),
        ),
        _State(  # 2: steady — 1 element/cycle; page wrap -> bubbles
            placement=p,
            trigger=(Trigger.SRC_TENSOR_DONE, Trigger.SUB_DIM_DONE, Trigger.NONE),
            next=(0, 3, 0), repeat=0, consume=(True, False), write_out=True,
        ),
        _State(  # 3: B1 — A <- A * w^PAGE (R held at st2)
            placement=p, trigger=COUNT_ONCE, repeat=1, consume=(False, False),
            overrides={
                0: _Stage(AluOp.BYPASS, CURR), 1: _BYP,
                2: _Stage(AluOp.BYPASS, CURR),
                3: _Stage(AluOp.MULTIPLY, CURR, PREV), 4: _BYP,
            },
            write_out=False, next=(4, 0, 0),
        ),
        _State(  # 4: B2 — reset Inv/R for the new page, hold A
            placement=p, trigger=COUNT_ONCE, repeat=1, consume=(False, False),
            overrides={
                0: _Stage(AluOp.BYPASS, One), 1: _BYP,
                2: _Stage(AluOp.BYPASS, One),
                3: _Stage(AluOp.BYPASS, CURR), 4: _BYP,
            },
            write_out=False, next=(2, 0, 0),
        ),
    ]
    uops = [_assemble(s) for s in states]
    for u in uops:
        u.validate("v3")
    return uops


def _ema_ref(in0, in1, s0, s1, imm2):
    P = in0.shape[0]
    u = in0.astype(np.float64).reshape(P, -1)
    w = np.asarray(s0, np.float64).reshape(P, 1)
    r = np.asarray(in1, np.float64).reshape(P, 1)[:, 0].copy()
    out = np.empty_like(u)
    for t in range(u.shape[1]):
        r = w[:, 0] * r + u[:, t]
        out[:, t] = r
    return out.reshape(in0.shape).astype(np.float32)


class _HandDveOp(DveOp):
    def compile(self, ver):
        if ver != "v3":
            raise ValueError(f"{self.name}: hand-built for v3/TRN2 only")
        cached = dve_ops._COMPILE_CACHE.get((self.name, ver))
        if cached is not None:
            return cached
        spec = DveOpSpec(
            name=self.name,
            opcode=dve_ops.get_dve_sub_opcode(self.name),
            uops=_build_ema_uops(),
            rd1_en=True,
        )
        dve_ops._COMPILE_CACHE[(self.name, ver)] = spec
        return spec


def _register_ema_op() -> DveOp:
    for op in dve_ops.OPS:
        if op.name == "EMA_PAGED_ANT":
            return op
    op = _HandDveOp(
        "EMA_PAGED_ANT",
        Spec(
            body=Bin(AluOp.ADD, Bin(AluOp.MULTIPLY, Src0, C1),
                     Bin(AluOp.MULTIPLY, Src1, C0)),
            reference=_ema_ref,
        ),
        subdim=True,
        uops_sha={},
    )
    dve_ops.OPS.append(op)
    dve_ops.CUSTOM_DVE_SPECS[op.name] = op.spec
    dve_ops._SUB_OPCODE_FOR_NAME[op.name] = (
        dve_ops._CUSTOM_DVE_ROW_BASE + len(dve_ops.OPS) - 1
    )
    assert dve_ops._SUB_OPCODE_FOR_NAME[op.name] < 0x20
    return op


# ---------------------------------------------------------------------------
# Kernel
# ---------------------------------------------------------------------------

B, T, D = 16, 4096, 512
NCORES = 8
BL = B // NCORES   # batches per core
P = 128            # partitions
TCW = 2048         # time chunk per pipeline iteration
ND = D // P        # channel chunks of 128 (4)
NK = TCW // P      # 128-row sub-chunks per time chunk (16)

FP32 = mybir.dt.float32
BF16 = mybir.dt.bfloat16


def build_program(bl: int = BL, t: int = T) -> bacc.Bacc:
    ema = _register_ema_op()
    nc = bacc.Bacc(
        "TRN2",
        target_bir_lowering=False,
        debug=False,
        enable_asserts=False,
        num_devices=NCORES,
    )
    x = nc.dram_tensor("x", (bl, t, D), BF16, kind="ExternalInput").ap()
    # Host-precomputed per-channel tables: (w | 1/w | 1/a | a), channel
    # d = j*128 + p laid out as [128, 4*ND].  Avoids the sigmoid
    # activation-table-load + reciprocal chain at startup.
    wt = nc.dram_tensor("wt", (P, 4 * ND), FP32, kind="ExternalInput").ap()
    y = nc.dram_tensor("y", (bl, t, D), BF16, kind="ExternalOutput").ap()

    with tile.TileContext(nc) as tc, ExitStack() as ctx:
        const_pool = ctx.enter_context(tc.tile_pool(name="const", bufs=1))
        xn_pool = ctx.enter_context(tc.tile_pool(name="xn", bufs=3))
        pin_pool = ctx.enter_context(tc.tile_pool(name="pin", bufs=2, space="PSUM"))
        pout_pool = ctx.enter_context(tc.tile_pool(name="pout", bufs=2, space="PSUM"))
        s_pool = ctx.enter_context(tc.tile_pool(name="s", bufs=20))
        y_pool = ctx.enter_context(tc.tile_pool(name="y", bufs=3))
        carry_pool = ctx.enter_context(tc.tile_pool(name="carry", bufs=1))

        ident = const_pool.tile([P, P], BF16)
        make_identity(nc, ident[:])

        # Per-channel tables ride the Scalar HWDGE queue so the Sync queue
        # starts streaming x immediately.
        wt_sb = const_pool.tile([P, 4 * ND], FP32)
        nc.scalar.dma_start(wt_sb[:], wt)
        w_sb = wt_sb[:, 0:ND]
        inv_w = wt_sb[:, ND : 2 * ND]
        inv_a = wt_sb[:, 2 * ND : 3 * ND]
        a_sb = wt_sb[:, 3 * ND : 4 * ND]

        # diag(a) per channel chunk: ident row p scaled by a[p] (bf16), on
        # the scalar engine (cheap per-partition scale).
        diags = []
        for j in range(ND):
            dg = const_pool.tile([P, P], BF16, tag=f"diag{j}")
            nc.scalar.mul(dg[:], ident[:], a_sb[:, j : j + 1])
            diags.append(dg)

        inits = carry_pool.tile([P, bl * ND], FP32)

        def back_chunk(ss, b, t0, m0, m1, yout, evac_engines=(nc.scalar,)):
            """Back-transpose (diag(a) matmul) + PSUM evac for k-pairs
            m0..m1 of the chunk whose scan outputs are `ss`."""
            for m in range(m0, m1):
                pout = pout_pool.tile([P, 2 * D], FP32, tag="pout")
                for h in range(2):
                    k = 2 * m + h
                    for j in range(ND):
                        nc.tensor.matmul(
                            pout[:, h * D + j * P : (h * D + (j + 1) * P)],
                            ss[j][:, k * P : (k + 1) * P],
                            diags[j][:],
                        )
                eng = evac_engines[m % len(evac_engines)]
                if eng is nc.vector:
                    nc.vector.tensor_copy(yout[:, 2 * m : 2 * m + 2, :], pout[:])
                else:
                    eng.copy(yout[:, 2 * m : 2 * m + 2, :], pout[:])

        def out_dma(eng, b, t0, r0, r1, yout):
            eng.dma_start(
                y[b, t0 + r0 : t0 + r1, :].rearrange("(k p) d -> p k d", p=P),
                yout[:, r0 // P : r1 // P, :],
            )

        # Chunk schedule: (b, t0, clen).  Geometric ramp: the early chunks
        # are small so the vector engine starts on the first 256KB of x and
        # keeps pace with the input-DMA ramp; the tensor engine then stays
        # continuously busy, which also warms the PE HAM clock-gate early.
        chunks = [
            (0, 0, 256),
            (1, 0, 256),
            (0, 256, 768),
            (1, 256, 768),
            (0, 1024, 1024),
            (1, 1024, 1024),
            (0, 2048, 2048),
            (1, 2048, 2048),
        ]
        s_prevs = [None] * bl
        pending = None  # deferred back-pass: (ss, b, t0, clen)
        for ci, (b, t0, clen) in enumerate(chunks):
            last = ci == len(chunks) - 1
            nk = clen // P
            xn = xn_pool.tile([P, NK, D], BF16, tag="xn")
            # Input pieces (Sync HWDGE, input-only so never blocked):
            if clen <= 768:
                pieces = (0, clen)
            else:
                pieces = (0, clen // 2, clen)
            for r0, r1 in zip(pieces[:-1], pieces[1:]):
                nc.sync.dma_start(
                    xn[:, r0 // P : r1 // P, :],
                    x[b, t0 + r0 : t0 + r1, :].rearrange("(k p) d -> p k d", p=P),
                )

            ss = []
            scan_args = []
            for j in range(ND):
                pin = pin_pool.tile([P, TCW], BF16, tag="pin")
                for k in range(nk):
                    nc.tensor.transpose(
                        pin[:, k * P : (k + 1) * P],
                        xn[:, k, j * P : (j + 1) * P],
                        ident[:],
                    )
                if t0 == 0:
                    # carry r_{-1} = x_0 / a  =>  s_0 = x_0 exactly
                    nc.scalar.mul(
                        inits[:, b * ND + j : b * ND + j + 1],
                        pin[:, 0:1],
                        inv_a[:, j : j + 1],
                    )
                    carry = inits[:, b * ND + j : b * ND + j + 1]
                else:
                    prev_s, prev_len = s_prevs[b]
                    carry = prev_s[j][:, prev_len - 1 : prev_len]
                s = s_pool.tile([P, TCW], BF16, tag="s", name=f"s_{b}_{t0}_{j}")
                ss.append(s)
                scan_args.append((pin, carry, s))

            def emit_scans(c0, c1):
                for j, (pin, carry, s) in enumerate(scan_args):
                    cr = carry if c0 == 0 else s[:, c0 - 1 : c0]
                    nc.vector._custom_dve(
                        ema,
                        out=s[:, c0:c1].rearrange("p (s n) -> p s n", n=PAGE),
                        in0=pin[:, c0:c1].rearrange("p (s n) -> p s n", n=PAGE),
                        in1=cr,
                        s0=w_sb[:, j : j + 1],
                        s1=inv_w[:, j : j + 1],
                    )

            if not last:
                emit_scans(0, clen)
                s_prevs[b] = (ss, clen)
                # Deferred back-pass of the PREVIOUS chunk: its scans are done
                # by now, so the in-order tensor queue never stalls ahead of
                # vector-critical transposes.
                if pending is not None:
                    pss, pb, pt0, pclen = pending
                    pnk = pclen // P
                    yout = y_pool.tile([P, NK, D], BF16, tag="y")
                    back_chunk(pss, pb, pt0, 0, pnk // 2, yout)
                    if pclen <= 512:
                        out_dma(nc.gpsimd if ci % 2 else nc.scalar, pb, pt0, 0, pclen, yout)
                    else:
                        out_dma(nc.gpsimd, pb, pt0, 0, pclen // 2, yout)
                        out_dma(nc.scalar, pb, pt0, pclen // 2, pclen, yout)
                pending = (ss, b, t0, clen)
            else:
                # Last chunk: scans in 512-step quarters with the back-pass
                # chasing each quarter; the final quarter's evacs run on the
                # (by then idle) vector engine and the last store rides the
                # (by then idle) Sync HWDGE queue, so the tail drains fast.
                Q = 512
                pss, pb, pt0, pclen = pending
                pnk = pclen // P
                pyout = y_pool.tile([P, NK, D], BF16, tag="y")
                yout = y_pool.tile([P, NK, D], BF16, tag="y")
                for q in range(4):
                    emit_scans(q * Q, (q + 1) * Q)
                    if q == 0:
                        # Previous chunk's deferred back-pass.
                        back_chunk(pss, pb, pt0, 0, pnk // 2, pyout)
                        out_dma(nc.gpsimd, pb, pt0, 0, pclen // 2, pyout)
                        out_dma(nc.scalar, pb, pt0, pclen // 2, pclen, pyout)
                    else:
                        m0, m1 = (q - 1) * 2, q * 2
                        back_chunk(ss, b, t0, m0, m1, yout)
                back_chunk(ss, b, t0, 6, 8, yout,
                           evac_engines=(nc.vector,))
                out_dma(nc.gpsimd, b, t0, 0, clen // 2, yout)
                out_dma(nc.scalar, b, t0, clen // 2, clen * 3 // 4, yout)
                out_dma(nc.sync, b, t0, clen * 3 // 4, clen, yout)

    nc.compile()
    return nc


_prog = None


def make_in_maps(x, alpha):
    x = np.asarray(x)
    alpha = np.asarray(alpha, dtype=np.float64)
    assert x.shape == (B, T, D) and alpha.shape == (1, 1, D)
    xb = np.ascontiguousarray(x.astype(ml_dtypes.bfloat16))
    # Per-channel tables (w | 1/w | 1/a | a) as [128, 4*ND], channel
    # d = j*128 + p (so column j holds channels j*128..j*128+127).
    a = 1.0 / (1.0 + np.exp(-alpha[0, 0, :]))  # (D,)
    w = 1.0 - a
    wt = np.empty((P, 4 * ND), np.float32)
    for bi, tb in enumerate((w, 1.0 / w, 1.0 / a, a)):
        wt[:, bi * ND : (bi + 1) * ND] = tb.reshape(ND, P).T
    return [
        {"x": np.ascontiguousarray(xb[i * BL : (i + 1) * BL]), "wt": wt}
        for i in range(NCORES)
    ]


def kernel(x, alpha):
    global _prog
    if _prog is None:
        _prog = build_program()
    in_maps = make_in_maps(x, alpha)
    res = run_bass_kernel_spmd(_prog, in_maps, core_ids=list(range(NCORES)))
    out = np.concatenate([r["y"] for r in res.results], axis=0)
    return np.ascontiguousarray(out.astype(np.float32))
